# revision 11
# baseline (speedup 1.0000x reference)
"""Single-head attention (B=8, T=2048, E=1024, D=128) on 8 Trainium2 NeuronCores.

Strategy (data-parallel over batch, one batch element per core):
  host: pre-transpose x -> xT[b] = x[b].T (E on rows) so the device needs no
        large transposes; all PE operands fp16 (full-rate matmuls).
  device, per core — a single quarter-interleaved pipeline:
    - for each T-quarter h: project k,v,q for that quarter (PE, accumulate
      over E in PSUM), drain with bias (+D**-0.25 scale for q,k) on DVE,
      transpose the vT quarter into V[k,d] (PE), and immediately compute
      span-0 scores for the two key-block groups of quarter h so the ACT
      engine starts exp work ~35us earlier than a phase-split schedule.
    - scores are computed two key blocks at a time into a 2-bank PSUM tile
      [128, 1024] so each exp is ONE activation instruction (halves ACT's
      fixed per-instruction overhead, the span-phase bottleneck).
    - softmax denominators: no ones-matmul on PE.  P groups are accumulated
      on DVE in fp16 (4x mode) into a per-span acc tile; the host sums the
      128 partitions and divides (normalization entirely off-device).
    - attention output OT[d,q] += V_blk.T @ P_blk accumulates in PSUM;
      span 0 accumulates per-quarter partials folded into SBUF on DVE.
    - x/weight input DMAs are split across the SP and Activation hwdge
      queues (a single queue cannot keep up with the PE) and interleaved
      in consumption order.
  host: out = (OT / denom).T per batch element.
"""

import os
import sys

for _p in ("/opt/trn_rl_repo",):
    if _p not in sys.path and os.path.isdir(_p):
        sys.path.append(_p)

import numpy as np

import concourse.bass as bass
import concourse.tile as tile
from concourse import mybir
from concourse.vector_clock import ScopedClock

B, T, E, D = 8, 2048, 1024, 128
EC = E // 128          # E chunks of 128 partitions
NSPAN = 4              # query spans of 512
SPAN = T // NSPAN      # 512
NKB = T // 128         # 16 key blocks
NG = NKB // 2          # 8 key-block groups (2 blocks / exp) per span
F32 = mybir.dt.float32
F32R = mybir.dt.float32r
BF16 = mybir.dt.bfloat16
F16 = mybir.dt.float16

_MAX_DRAIN_WAITS = 1


def _drain_and_barrier_split(self, tick_clock, wait_clock):
    # This walrus build rejects CTRL instructions carrying more than one sync
    # wait, so spread the kernel-tail drain's waits over single-wait NOPs.
    nc = self.nc
    collector = nc.sync.nop(nofuse=True, hint="drain_wait_collector")
    wait_clock.add_sem_waits(
        collector.ins, ScopedClock({None: tick_clock.global_clock})
    )
    si = collector.ins.sync_info
    waits = list(si.on_wait) if si and si.on_wait else []
    if len(waits) > _MAX_DRAIN_WAITS:
        si.on_wait = waits[:_MAX_DRAIN_WAITS]
        rest = waits[_MAX_DRAIN_WAITS:]
        while rest:
            chunk, rest = rest[:_MAX_DRAIN_WAITS], rest[_MAX_DRAIN_WAITS:]
            extra = nc.sync.nop(nofuse=True, hint="drain_wait_extra")
            if extra.ins.sync_info is None:
                extra.ins.sync_info = type(si)(on_wait=chunk, on_update=[])
            else:
                extra.ins.sync_info.on_wait = chunk

    nc.sync.drain()

    nc.all_engine_barrier()
    assert self.sems is not None
    popped = nc._tile_sem_poison_stack.pop()
    assert popped is self._sem_poison
    nc.clear_and_free_semaphores(list(self.sems.allocated().values()))
    nc.all_engine_barrier()


tile.TileContext._drain_and_barrier = _drain_and_barrier_split


def _split_excess_waits(nc):
    """Walrus in this env allows at most one sync wait per instruction;
    hoist extra waits onto same-engine NOPs placed just before."""
    import copy

    m = nc.m
    cnt = 0
    new_funcs = []
    for function in m.functions:
        new_function = copy.replace(function, blocks=[])
        new_function.set_allocations_from_list(function.allocations)
        for block in function.blocks:
            new_insts = []
            for inst in block.instructions:
                si = inst.sync_info
                waits = list(si.on_wait) if si and si.on_wait else []
                if len(waits) > 1:
                    for w in waits[:-1]:
                        nop = mybir.InstNoOp(name=f"I-swsplit-{cnt}",
                                             ins=[], outs=[])
                        cnt += 1
                        nop.engine = inst.engine
                        nop.sync_info = mybir.SyncInfo(on_wait=[w],
                                                       on_update=[])
                        new_insts.append(nop)
                    si.on_wait = [waits[-1]]
                new_insts.append(inst)
            new_function.blocks.append(
                copy.replace(block, instructions=new_insts))
        new_funcs.append(new_function)
    new_m = copy.replace(m, functions=[])
    for f in new_funcs:
        new_m.functions.append(f)
    nc.m = new_m
    return cnt


def build_nc():
    SCALE = float(np.float32(D) ** np.float32(-0.25))
    ADD = mybir.AluOpType.add
    MULT = mybir.AluOpType.mult
    EXP = mybir.ActivationFunctionType.Exp

    nc = bass.Bass()
    xT = nc.declare_dram_parameter("xT", [E, T], F16, isOutput=False)[:]
    Wq = nc.declare_dram_parameter("Wq", [128, EC * D], F16, isOutput=False)[:]
    Wk = nc.declare_dram_parameter("Wk", [128, EC * D], F16, isOutput=False)[:]
    Wv = nc.declare_dram_parameter("Wv", [128, EC * D], F16, isOutput=False)[:]
    bqc = nc.declare_dram_parameter("bqc", [D], F32, isOutput=False)[:]
    bkc = nc.declare_dram_parameter("bkc", [D], F32, isOutput=False)[:]
    bv = nc.declare_dram_parameter("bv", [D], F32, isOutput=False)[:]
    ident_d = nc.declare_dram_parameter("ident", [128, 128], F16,
                                        isOutput=False)[:]
    outT = nc.declare_dram_parameter("outT", [D, T], F16, isOutput=True)[:]
    accT = nc.declare_dram_parameter("accT", [128, T], F16, isOutput=True)[:]

    wq_r = Wq.rearrange("p (c d) -> p c d", d=D)
    wk_r = Wk.rearrange("p (c d) -> p c d", d=D)
    wv_r = Wv.rearrange("p (c d) -> p c d", d=D)

    with tile.TileContext(nc) as tc, \
         tc.tile_pool(name="consts", bufs=1) as consts, \
         tc.tile_pool(name="xpool", bufs=1) as xpool, \
         tc.tile_pool(name="persist", bufs=1) as persist, \
         tc.tile_pool(name="ppool", bufs=8) as ppool, \
         tc.tile_pool(name="outpool", bufs=3) as outpool, \
         tc.tile_pool(name="psS", bufs=2, space="PSUM") as psS, \
         tc.tile_pool(name="psB", bufs=3, space="PSUM") as psB:

        # ---- SBUF tiles ----
        wq_s = consts.tile([128, EC, D], F16, tag="wq")
        wk_s = consts.tile([128, EC, D], F16, tag="wk")
        wv_s = consts.tile([128, EC, D], F16, tag="wv")
        bq_s = consts.tile([128, 1], F32, tag="bq")
        bk_s = consts.tile([128, 1], F32, tag="bk")
        bv_s = consts.tile([128, 1], F32, tag="bv")
        ident = consts.tile([128, 128], F16, tag="ident")

        kT_s = persist.tile([128, T], F16, tag="kT")
        vT_s = persist.tile([128, T], F16, tag="vT")
        qT_s = persist.tile([128, T], F16, tag="qT")
        V_s = persist.tile([128, NKB, D], F16, tag="V")
        accO = persist.tile([128, SPAN], F32, tag="accO")
        acc2 = [persist.tile([128, 1024], F16, tag=f"acc{s}", name=f"acc{s}")
                for s in range(NSPAN)]

        xp = [[xpool.tile([128, SPAN], F16, tag=f"xp{e}_{h}",
                          name=f"xp{e}_{h}")
               for h in range(4)] for e in range(EC)]

        # ---- input DMAs, split across the SP and ACT hwdge queues,
        # ordered to land just ahead of PE consumption (x first: each
        # dma_start costs ~650ns of issue time on its engine) ----
        def xsl(e):
            return slice(e * 128, (e + 1) * 128)

        for b_s, b_d in ((bq_s, bqc), (bk_s, bkc), (bv_s, bv)):
            nc.gpsimd.dma_start(out=b_s, in_=b_d.unsqueeze(1))
        nc.sync.dma_start(out=ident, in_=ident_d)
        nc.sync.dma_start(out=wk_s, in_=wk_r)
        nc.scalar.dma_start(out=xp[1][0], in_=xT[xsl(1), 0:SPAN])
        nc.scalar.dma_start(out=xp[3][0], in_=xT[xsl(3), 0:SPAN])
        nc.sync.dma_start(out=xp[0][0], in_=xT[xsl(0), 0:SPAN])
        nc.scalar.dma_start(out=xp[5][0], in_=xT[xsl(5), 0:SPAN])
        nc.sync.dma_start(out=xp[2][0], in_=xT[xsl(2), 0:SPAN])
        nc.scalar.dma_start(out=xp[7][0], in_=xT[xsl(7), 0:SPAN])
        nc.sync.dma_start(out=xp[4][0], in_=xT[xsl(4), 0:SPAN])
        nc.scalar.dma_start(out=wv_s, in_=wv_r)
        nc.sync.dma_start(out=xp[6][0], in_=xT[xsl(6), 0:SPAN])
        nc.scalar.dma_start(out=wq_s, in_=wq_r)
        for h in range(1, 4):
            hsl = slice(h * SPAN, (h + 1) * SPAN)
            for e in range(EC):
                eng = nc.sync if e % 2 == 0 else nc.scalar
                eng.dma_start(out=xp[e][h], in_=xT[xsl(e), hsl])

        # ---- PE warmup: transpose-spin on ident while x streams in, so
        # the tensor engine is at full p-state when projections start ----
        warm = psS.tile([128, SPAN], F16, tag="sm", name="warm")
        for w in range(14):
            nc.tensor.transpose(warm[:, 0:128], ident, ident)

        # ---- helpers ----
        p_tiles = {}
        deferred_add = []

        def scores_grp(s, g, split_exp=False):
            """Scores for key blocks (2g, 2g+1) vs query span s, one
            2-bank PSUM tile, one batched exp, one DVE P-accumulate."""
            st = psB.tile([128, 1024], F32, tag="big", name="st")
            ssl = slice(s * SPAN, (s + 1) * SPAN)
            for j in (0, 1):
                kb = 2 * g + j
                nc.tensor.matmul(st[:, j * 512:(j + 1) * 512],
                                 kT_s[:, kb * 128:(kb + 1) * 128],
                                 qT_s[:, ssl], start=True, stop=True)
            p = ppool.tile([128, 1024], F16, tag="p", name="p")
            if split_exp:
                nc.scalar.activation(out=p[:, 0:512], in_=st[:, 0:512],
                                     func=EXP)
                nc.scalar.activation(out=p[:, 512:1024], in_=st[:, 512:1024],
                                     func=EXP)
            else:
                nc.scalar.activation(out=p, in_=st, func=EXP)
            if split_exp:
                deferred_add.append((s, p))  # emit after the output copies
            elif g == 0:
                nc.vector.tensor_copy(out=acc2[s], in_=p)
            else:
                nc.vector.tensor_add(out=acc2[s], in0=acc2[s], in1=p)
            p_tiles[(s, g)] = p

        def av_grp(s, g, ot, start, stop):
            p = p_tiles.pop((s, g))
            for j in (0, 1):
                kb = 2 * g + j
                nc.tensor.matmul(ot, V_s[:, kb, :],
                                 p[:, j * 512:(j + 1) * 512],
                                 start=start and j == 0,
                                 stop=stop and j == 1)

        def acc_out(s, eng=None):
            """Fold the two P-sum halves and DMA the span's denominator
            partials (host sums the 128 partitions)."""
            accf = outpool.tile([128, SPAN], F16, tag="accf", name="accf")
            nc.vector.tensor_add(out=accf, in0=acc2[s][:, 0:512],
                                 in1=acc2[s][:, 512:1024])
            (eng or nc.sync).dma_start(
                out=accT[:, s * SPAN:(s + 1) * SPAN], in_=accf)

        # ---- quarter-interleaved projections + span-0 attention ----
        for h in range(4):
            hsl = slice(h * SPAN, (h + 1) * SPAN)

            k_ps = psS.tile([128, SPAN], F32, tag="sm", name="k_ps")
            for e in range(EC):
                nc.tensor.matmul(k_ps, wk_s[:, e, :], xp[e][h],
                                 start=(e == 0), stop=(e == EC - 1))
            nc.vector.tensor_scalar(out=kT_s[:, hsl], in0=k_ps,
                                    scalar1=bk_s, scalar2=SCALE,
                                    op0=ADD, op1=MULT)

            v_ps = psS.tile([128, SPAN], F32, tag="sm", name="v_ps")
            for e in range(EC):
                nc.tensor.matmul(v_ps, wv_s[:, e, :], xp[e][h],
                                 start=(e == 0), stop=(e == EC - 1))
            nc.vector.tensor_scalar(out=vT_s[:, hsl], in0=v_ps,
                                    scalar1=bv_s, scalar2=None, op0=ADD)

            q_ps = psS.tile([128, SPAN], F32, tag="sm", name="q_ps")
            for e in range(EC):
                nc.tensor.matmul(q_ps, wq_s[:, e, :], xp[e][h],
                                 start=(e == 0), stop=(e == EC - 1))
            nc.vector.tensor_scalar(out=qT_s[:, hsl], in0=q_ps,
                                    scalar1=bq_s, scalar2=SCALE,
                                    op0=ADD, op1=MULT)

            # attention-output partials for the previous quarter's span-0
            # groups (their exps completed while this quarter projected)
            if h >= 1:
                otq = psS.tile([128, SPAN], F32, tag="sm", name="otq")
                av_grp(0, 2 * (h - 1), otq, start=True, stop=False)
                av_grp(0, 2 * h - 1, otq, start=False, stop=True)
                if h == 1:
                    nc.vector.tensor_copy(out=accO, in_=otq)
                else:
                    nc.vector.tensor_add(out=accO, in0=accO, in1=otq)

            # V blocks for this quarter (PE transpose of vT, one cast)
            vt_ps = psS.tile([128, SPAN], F16, tag="sm", name="vt_ps")
            for j in range(4):
                kb = 4 * h + j
                nc.tensor.transpose(vt_ps[:, j * 128:(j + 1) * 128],
                                    vT_s[:, kb * 128:(kb + 1) * 128], ident)
            nc.vector.tensor_copy(out=V_s[:, 4 * h:4 * h + 4, :], in_=vt_ps)

            scores_grp(0, 2 * h)
            scores_grp(0, 2 * h + 1)
            if h >= 2:
                scores_grp(1, 2 * (h - 2))
                scores_grp(1, 2 * (h - 2) + 1)

        # ---- remaining spans, scores emitted ahead of AVs (FIFO) ----
        rest = [(1, g) for g in range(4, NG)] + \
               [(s, g) for s in (2, 3) for g in range(NG)]
        pend = [(1, 0), (1, 1), (1, 2), (1, 3)]
        ot_cur = {}

        def av_rest(s, g):
            if g == 0:
                ot_cur[s] = psS.tile([128, SPAN], F32, tag="sm", name="ot")
            av_grp(s, g, ot_cur[s], start=(g == 0), stop=(g == NG - 1))
            if g == NG - 1:
                osb = outpool.tile([128, SPAN], F16, tag="osb", name="osb")
                osl = slice(s * SPAN, (s + 1) * SPAN)
                if s == 3:
                    # pipelined half-drain of the final span
                    nc.vector.tensor_copy(out=osb[:, 0:256],
                                          in_=ot_cur[s][:, 0:256])
                    nc.sync.dma_start(out=outT[:, osl][:, 0:256],
                                      in_=osb[:, 0:256])
                    nc.vector.tensor_copy(out=osb[:, 256:512],
                                          in_=ot_cur[s][:, 256:512])
                    nc.sync.dma_start(out=outT[:, osl][:, 256:512],
                                      in_=osb[:, 256:512])
                    for ds, dp in deferred_add:
                        nc.vector.tensor_add(out=acc2[ds], in0=acc2[ds],
                                             in1=dp)
                else:
                    nc.vector.tensor_copy(out=osb, in_=ot_cur[s])
                    nc.sync.dma_start(out=outT[:, osl], in_=osb)
                acc_out(s, eng=nc.scalar if s >= 2 else None)

        for idx, (s, g) in enumerate(rest):
            scores_grp(s, g, split_exp=s == 3 and g >= NG - 2)
            pend.append((s, g))
            if idx == 1:
                # span 0 epilogue: last AV pair, fold, outputs
                otq = psS.tile([128, SPAN], F32, tag="sm", name="otq3")
                av_grp(0, 6, otq, start=True, stop=False)
                av_grp(0, 7, otq, start=False, stop=True)
                nc.vector.tensor_add(out=accO, in0=accO, in1=otq)
                osb0 = outpool.tile([128, SPAN], F16, tag="osb", name="osb0")
                nc.vector.tensor_copy(out=osb0, in_=accO)
                nc.sync.dma_start(out=outT[:, 0:SPAN], in_=osb0)
                acc_out(0)
            elif idx >= 2:
                av_rest(*pend.pop(0))
        while pend:
            av_rest(*pend.pop(0))

    return nc


_CACHED = {}


def _get_nc(key="f16"):
    if key not in _CACHED:
        nc = build_nc()
        _split_excess_waits(nc)
        _CACHED[key] = nc
    return _CACHED[key]


def _make_in_maps(x, Wq, bq, Wk, bk, Wv, bv):
    def rnd(a):
        return np.ascontiguousarray(np.asarray(a, np.float32), np.float16)

    xT = rnd(np.transpose(np.asarray(x, np.float32), (0, 2, 1)))

    def warr(w):
        w = np.asarray(w, np.float32).reshape(EC, 128, D)
        return rnd(w.transpose(1, 0, 2).reshape(128, EC * D))

    Wq, Wk, Wv = warr(Wq), warr(Wk), warr(Wv)
    bqc = np.ascontiguousarray(np.asarray(bq, np.float32))
    bkc = np.ascontiguousarray(np.asarray(bk, np.float32))
    bv = np.ascontiguousarray(np.asarray(bv, np.float32))
    ident = np.eye(128, dtype=np.float16)
    return [
        {"xT": np.ascontiguousarray(xT[b]), "Wq": Wq, "Wk": Wk, "Wv": Wv,
         "bqc": bqc, "bkc": bkc, "bv": bv, "ident": ident}
        for b in range(B)
    ]


def kernel(x, Wq, bq, Wk, bk, Wv, bv, _trace=False, _mm_dt=None):
    from concourse.bass_utils import run_bass_kernel_spmd

    nc = _get_nc()
    in_maps = _make_in_maps(x, Wq, bq, Wk, bk, Wv, bv)
    res = run_bass_kernel_spmd(nc, in_maps, core_ids=list(range(B)),
                               trace=_trace)
    out = np.empty((B, T, D), np.float32)
    for b in range(B):
        ot = np.asarray(res.results[b]["outT"]).astype(np.float32)
        ac = np.asarray(res.results[b]["accT"]).astype(np.float32)
        denom = ac.sum(axis=0)                                   # [T]
        out[b] = (ot / denom[None, :]).T
    kernel._last_result = res
    return out
